# revision 18
# baseline (speedup 1.0000x reference)
"""Trainium2 Bass kernel for the CounterPrimitive module.

Computes, for x (B=4, L=8192, D=1024):
  inc_logits   = x @ inc_w + inc_b                     (B, L, 2)
  counters     = scan(c = keep*c + inc) over L         (B, L, 2)
  injection    = [sin|cos](counters/freqs) @ read_w + read_b   (B, L, 1024)

Sharding: 8 cores = (batch b, L-half h).  Each core handles a (4096, 1024)
x shard.  The scan carry between the two L-halves of a batch (2 floats) is
exchanged with a paired AllReduce.
"""

import math
import os
from contextlib import ExitStack

import numpy as np

import concourse.bacc as bacc
import concourse.bass as bass
import concourse.mybir as mybir
import concourse.tile as tile
from concourse.bass_utils import run_bass_kernel_spmd

AF = mybir.ActivationFunctionType
ALU = mybir.AluOpType
DT = mybir.dt.float32

B, L, D, K = 4, 8192, 1024, 2
N_FREQS = 16
MAX_PERIOD = 4096.0
N_CORES = 8
LS = L // 2            # per-core L shard
NCH = 8                # 512-wide L chunks per core
CH = 512
NBLK = 64              # scan blocks per k channel
BJ = 64                # elements per scan block

_CACHE = {}


def build_program(timing_variant=False):
    # timing_variant: single-core, collective replaced by a local DMA copy —
    # for TimelineSim cost-model analysis only (TimelineSim is 1-core).
    nc = bacc.Bacc("TRN2", target_bir_lowering=False, debug=False,
                   num_devices=1 if timing_variant else N_CORES)

    xs = nc.dram_tensor("xs", [LS, D], DT, kind="ExternalInput").ap()
    w4 = nc.dram_tensor("w4", [128, 32], DT, kind="ExternalInput").ap()
    rw = nc.dram_tensor("rw", [65, D], DT, kind="ExternalInput").ap()
    bias4 = nc.dram_tensor("bias4", [4, 1], DT, kind="ExternalInput").ap()
    ident = nc.dram_tensor("ident", [128, 128], DT, kind="ExternalInput").ap()
    sel = nc.dram_tensor("sel", [3, 64], DT, kind="ExternalInput").ap()
    evenm = nc.dram_tensor("evenm", [1, 2], DT, kind="ExternalInput").ap()
    oddm = nc.dram_tensor("oddm", [1, 2], DT, kind="ExternalInput").ap()

    inj = nc.dram_tensor("inj", [LS, D], DT, kind="ExternalOutput").ap()
    lgT = nc.dram_tensor("lgT", [2, LS], DT, kind="ExternalOutput").ap()
    cntT = nc.dram_tensor("cntT", [128, BJ], DT, kind="ExternalOutput").ap()

    groups = [[0, 1], [2, 3], [4, 5], [6, 7]]

    with tile.TileContext(nc) as tc, ExitStack() as ctx:
        consts = ctx.enter_context(tc.tile_pool(name="consts", bufs=1))
        gbuf = ctx.enter_context(tc.tile_pool(name="gbuf", bufs=1))
        dram = ctx.enter_context(tc.tile_pool(name="dram", bufs=1, space="DRAM"))

        w4_sb = consts.tile([128, 32], DT)
        nc.sync.dma_start(w4_sb[:], w4)
        rw_sb = consts.tile([65, D], DT)
        nc.sync.dma_start(rw_sb[:], rw)
        bias4_sb = consts.tile([4, 1], DT)
        nc.sync.dma_start(bias4_sb[:], bias4)
        ident_sb = consts.tile([128, 128], DT)
        nc.sync.dma_start(ident_sb[:], ident)
        sel_sb = consts.tile([3, 64], DT)
        nc.sync.dma_start(sel_sb[:], sel)
        evenm_sb = consts.tile([1, 2], DT)
        nc.sync.dma_start(evenm_sb[:], evenm)
        oddm_sb = consts.tile([1, 2], DT)
        nc.sync.dma_start(oddm_sb[:], oddm)

        # persistent buffers
        graw = gbuf.tile([4, LS], DT)         # gate logits, rows (ik0,ik1,rk0,rk1)
        inc_rl = gbuf.tile([128, BJ], DT)     # inc logits,  [(k,blk), j]
        reset_rl = gbuf.tile([128, BJ], DT)

        # ---------------- Phase A: gates ----------------
        with tc.tile_pool(name="phA", bufs=2) as phA, \
             tc.tile_pool(name="psA", bufs=2, space="PSUM") as psA:
            for ci in range(NCH):
                xt = phA.tile([128, 4, D], DT, tag="x", bufs=2)
                nc.sync.dma_start(
                    xt[:], xs[ci * CH:(ci + 1) * CH, :].rearrange(
                        "(s p) d -> p s d", p=128))
                gps = psA.tile([4, CH], DT, tag="g", bufs=2)
                for dc in range(8):
                    tps = psA.tile([128, CH], DT, tag="tp", bufs=2)
                    for s in range(4):
                        nc.tensor.transpose(
                            tps[:, s * 128:(s + 1) * 128],
                            xt[:, s, dc * 128:(dc + 1) * 128],
                            ident_sb[:])
                    xts = phA.tile([128, CH], DT, tag="xt", bufs=3)
                    nc.any.tensor_copy(xts[:], tps[:])
                    nc.tensor.matmul(gps[:], w4_sb[:, dc * 4:(dc + 1) * 4],
                                     xts[:], start=(dc == 0), stop=(dc == 7))
                # add gate biases, park logits in SBUF
                nc.scalar.activation(graw[:, ci * CH:(ci + 1) * CH], gps[:],
                                     AF.Identity, bias=bias4_sb[:])

        nc.sync.dma_start(lgT, graw[0:2, :])
        # redistribute logits rows -> [(k, blk), j] layout (via DRAM: SBUF APs
        # cannot reshape a free-dim run into partitions, DRAM is flat)
        glog_d = dram.tile([4, LS], DT)
        nc.sync.dma_start(glog_d[:], graw[:])
        for k in range(2):
            nc.sync.dma_start(
                inc_rl[k * 64:(k + 1) * 64, :],
                glog_d[k, :].rearrange("(b j) -> b j", j=BJ))
            nc.sync.dma_start(
                reset_rl[k * 64:(k + 1) * 64, :],
                glog_d[2 + k, :].rearrange("(b j) -> b j", j=BJ))

        # ---------------- Phase B: scan + cross-core carry ----------------
        with tc.tile_pool(name="psB", bufs=1, space="PSUM") as psB:
            inc_s = gbuf.tile([128, BJ], DT)
            keep = gbuf.tile([128, BJ], DT)
            nc.scalar.activation(inc_s[:], inc_rl[:], AF.Sigmoid)
            nc.scalar.activation(keep[:], reset_rl[:], AF.Sigmoid, scale=-1.0)

            zeros64 = gbuf.tile([128, BJ], DT)
            nc.vector.memset(zeros64[:], 0.0)
            c_loc = gbuf.tile([128, BJ], DT)
            pcum = gbuf.tile([128, BJ], DT)
            nc.vector.tensor_tensor_scan(c_loc[:], keep[:], inc_s[:], 0.0,
                                         ALU.mult, ALU.add)
            nc.vector.tensor_tensor_scan(pcum[:], keep[:], zeros64[:], 1.0,
                                         ALU.mult, ALU.add)

            # block summaries -> rows at partition 0 (DVE operands must start
            # at a x32 partition, so K_blk and I_blk get separate transposes)
            pstK = psB.tile([1, 128], DT, tag="smallK", bufs=1)
            nc.tensor.transpose(pstK[:], pcum[:, BJ - 1:BJ], ident_sb[:])
            pstI = psB.tile([1, 128], DT, tag="smallI", bufs=1)
            nc.tensor.transpose(pstI[:], c_loc[:, BJ - 1:BJ], ident_sb[:])
            sumK = gbuf.tile([1, 128], DT)
            nc.vector.tensor_copy(sumK[:], pstK[:])
            sumI = gbuf.tile([1, 128], DT)
            nc.vector.tensor_copy(sumI[:], pstI[:])

            # block-level scan (per k), zero carry first to get the send value
            crow = gbuf.tile([1, 128], DT)
            for k in range(2):
                sl = slice(k * 64, (k + 1) * 64)
                nc.vector.tensor_tensor_scan(crow[0:1, sl], sumK[0:1, sl],
                                             sumI[0:1, sl], 0.0,
                                             ALU.mult, ALU.add)
            ccin_sb = gbuf.tile([1, 2], DT)
            nc.vector.tensor_copy(ccin_sb[:, 0:1], crow[:, 63:64])
            nc.vector.tensor_copy(ccin_sb[:, 1:2], crow[:, 127:128])
            ccin_m = gbuf.tile([1, 2], DT)
            nc.vector.tensor_mul(ccin_m[:], ccin_sb[:], evenm_sb[:])

            ccin_d = dram.tile([1, 2], DT)
            ccout_d = dram.tile([1, 2], DT)
            nc.gpsimd.dma_start(ccin_d[:], ccin_m[:])
            if timing_variant:
                nc.gpsimd.dma_start(ccout_d[:], ccin_d[:])
            else:
                nc.gpsimd.collective_compute(
                    "AllReduce", ALU.add, replica_groups=groups,
                    ins=[ccin_d.opt()], outs=[ccout_d.opt()])
            ccres = gbuf.tile([1, 2], DT)
            nc.gpsimd.dma_start(ccres[:], ccout_d[:])
            carry = gbuf.tile([1, 2], DT)
            nc.vector.tensor_mul(carry[:], ccres[:], oddm_sb[:])

            # re-scan with the core carry, build exclusive block carries
            crow2 = gbuf.tile([1, 128], DT)
            for k in range(2):
                sl = slice(k * 64, (k + 1) * 64)
                nc.vector.tensor_tensor_scan(crow2[0:1, sl], sumK[0:1, sl],
                                             sumI[0:1, sl],
                                             carry[0:1, k:k + 1],
                                             ALU.mult, ALU.add)
            cex = gbuf.tile([1, 128], DT)
            for k in range(2):
                nc.vector.tensor_copy(cex[:, k * 64 + 1:(k + 1) * 64],
                                      crow2[:, k * 64:(k + 1) * 64 - 1])
                nc.vector.tensor_copy(cex[:, k * 64:k * 64 + 1],
                                      carry[:, k:k + 1])
            pbc = psB.tile([128, 1], DT, tag="small2", bufs=1)
            nc.tensor.transpose(pbc[:], cex[:], ident_sb[0:1, 0:1])
            bcar = gbuf.tile([128, 1], DT)
            nc.vector.tensor_copy(bcar[:], pbc[:])

            counters = gbuf.tile([128, BJ], DT)
            nc.vector.scalar_tensor_tensor(counters[:], pcum[:], bcar[:],
                                           c_loc[:], ALU.mult, ALU.add)
            nc.sync.dma_start(cntT, counters[:])

            # counters back to row layout for the embedding (via DRAM).
            # crowc row 0 = ones (bias row for the sel matmul), rows 1,2 = k0,k1
            cnt_d = dram.tile([2, LS], DT)
            nc.sync.dma_start(
                cnt_d[:].rearrange("k (b j) -> (k b) j", j=BJ), counters[:])
            crowc = gbuf.tile([3, LS], DT)
            nc.vector.memset(crowc[0:1, :], 1.0)
            for k in range(2):
                nc.sync.dma_start(crowc[k + 1:k + 2, :], cnt_d[k:k + 1, :])

        # ---------------- Phase C: embedding + injection ----------------
        with tc.tile_pool(name="phC", bufs=3) as phC, \
             tc.tile_pool(name="psC", bufs=2, space="PSUM") as psC:
            for ci in range(NCH):
                # cbp[e, l] = counters_k(e)[l]/freq[e] + trig[e]*pi/2   (>= 0)
                cbp = psC.tile([64, CH], DT, tag="cb", bufs=2)
                nc.tensor.matmul(cbp[:], sel_sb[:],
                                 crowc[:, ci * CH:(ci + 1) * CH],
                                 start=True, stop=True)
                # range reduction: w2 = frac-centered(t / 2pi) in [-0.5, 0.5),
                # robust to the fp->int cast rounding mode (trunc or RNE),
                # then Sin(2pi * w2).  (No mod/floor op exists on TRN2.)
                m = phC.tile([64, CH], DT, tag="m", bufs=2)
                nc.vector.tensor_scalar_mul(m[:], cbp[:], 1.0 / (2 * math.pi))
                mi = phC.tile([64, CH], mybir.dt.int32, tag="mi", bufs=2)
                nc.any.tensor_copy(mi[:], m[:])
                mf = phC.tile([64, CH], DT, tag="mf", bufs=2)
                nc.any.tensor_copy(mf[:], mi[:])
                w = phC.tile([64, CH], DT, tag="w", bufs=2)
                nc.vector.tensor_sub(w[:], m[:], mf[:])
                cge = phC.tile([64, CH], DT, tag="cge", bufs=2)
                nc.vector.tensor_scalar(cge[:], w[:], 0.5, None, ALU.is_ge)
                w2 = phC.tile([64, CH], DT, tag="w2", bufs=2)
                nc.vector.tensor_sub(w2[:], w[:], cge[:])
                embT = phC.tile([65, CH], DT, tag="embT", bufs=3)
                nc.scalar.activation(embT[0:64, :], w2[:], AF.Sin,
                                     scale=2 * math.pi)
                nc.vector.memset(embT[64:65, :], 1.0)
                for ls in range(4):
                    ip0 = psC.tile([128, CH], DT, tag="inj0", bufs=2)
                    ip1 = psC.tile([128, CH], DT, tag="inj1", bufs=2)
                    lhs = embT[:, ls * 128:(ls + 1) * 128]
                    nc.tensor.matmul(ip0[:], lhs, rw_sb[:, 0:CH],
                                     start=True, stop=True)
                    nc.tensor.matmul(ip1[:], lhs, rw_sb[:, CH:D],
                                     start=True, stop=True)
                    io = phC.tile([128, D], DT, tag="io", bufs=3)
                    nc.any.tensor_copy(io[:, 0:CH], ip0[:])
                    nc.any.tensor_copy(io[:, CH:D], ip1[:])
                    r0 = (ci * 4 + ls) * 128
                    nc.sync.dma_start(inj[r0:r0 + 128, :], io[:])

    nc.compile()
    return nc


def host_constants(inc_w, inc_b, reset_w, reset_b, read_w, read_b):
    f32 = np.float32
    w4_full = np.concatenate([inc_w, reset_w], axis=1).astype(f32)   # (D, 4)
    w4 = np.ascontiguousarray(
        w4_full.reshape(8, 128, 4).transpose(1, 0, 2).reshape(128, 32))
    rw = np.concatenate([read_w, read_b[None, :]], axis=0).astype(f32)
    freqs = np.exp(np.linspace(0.0, math.log(MAX_PERIOD), N_FREQS)).astype(f32)
    e = np.arange(64)
    invf = (1.0 / freqs[e % 16]).astype(f32)
    trig = (e // 16) % 2
    k_of_e = e // 32
    bias4 = np.array([inc_b[0], inc_b[1], reset_b[0], reset_b[1]],
                     dtype=f32).reshape(4, 1)
    ident = np.eye(128, dtype=f32)
    # sel row 0 multiplies the ones row (phase bias), rows 1,2 pick k0/k1
    selm = np.zeros((3, 64), dtype=f32)
    selm[0, :] = trig * (math.pi / 2)
    selm[1, :] = np.where(k_of_e == 0, invf, 0.0)
    selm[2, :] = np.where(k_of_e == 1, invf, 0.0)
    return dict(w4=w4, rw=rw, bias4=bias4, ident=ident, sel=selm)


def run(x, inc_w, inc_b, reset_w, reset_b, read_w, read_b, trace=False):
    if "nc" not in _CACHE:
        _CACHE["nc"] = build_program()
    nc = _CACHE["nc"]

    consts = host_constants(inc_w, inc_b, reset_w, reset_b, read_w, read_b)
    x = np.ascontiguousarray(x, dtype=np.float32)
    in_maps = []
    for c in range(N_CORES):
        b, h = c // 2, c % 2
        m = dict(consts)
        m["xs"] = np.ascontiguousarray(x[b, h * LS:(h + 1) * LS, :])
        m["evenm"] = np.full((1, 2), 1.0 - h, dtype=np.float32)
        m["oddm"] = np.full((1, 2), float(h), dtype=np.float32)
        in_maps.append(m)

    res = run_bass_kernel_spmd(nc, in_maps, core_ids=list(range(N_CORES)),
                               trace=trace)

    injection = np.empty((B, L, D), dtype=np.float32)
    inc_logits = np.empty((B, L, K), dtype=np.float32)
    counters = np.empty((B, L, K), dtype=np.float32)
    for c in range(N_CORES):
        b, h = c // 2, c % 2
        sl = slice(h * LS, (h + 1) * LS)
        r = res.results[c]
        injection[b, sl, :] = r["inj"]
        inc_logits[b, sl, :] = r["lgT"].T
        counters[b, sl, :] = (
            r["cntT"].reshape(2, NBLK * BJ).T)
    return (injection, inc_logits, counters), res


def kernel(**inputs):
    out, _ = run(**inputs, trace=os.environ.get("KERNEL_TRACE") == "1")
    return out
